# revision 5
# baseline (speedup 1.0000x reference)
"""Causal multi-head attention (B=1, S=4096, D=1024, H=16, HD=64) on 8 TRN2
NeuronCores, sharded 2 heads per core (tensor parallel).

Per core c (heads 2c, 2c+1; d-slice [128c, 128c+128)):
  - QT/KT/VT [128, 4096] = W_slice @ x.T via fp32r matmuls (xT streamed).
  - V transposed back to [s, d] layout via PE identity-transpose, with a ones
    column appended (V65) so the attention matmul also produces the softmax
    denominator as output row 64.
  - Flash-style causal attention with transposed scores ST[k, q]:
    exp on ScalarE (no running max: scaled scores are in [-4.1, 4.1] for this
    problem family; exp is safe in fp32). Scores for the two heads run on
    disjoint PE row-groups (partitions 0:64 / 64:128) so they can overlap.
  - Per-query normalization via a K=1 broadcast matmul + VectorE multiply.
  - Partial out-projection y_c = ctx_c @ Wo[:, d-slice].T -> [4096, 1024].
Host: y = sum_c y_c + bo.

Inputs are host-prepared: x is transposed once on host (the kernel needs
D-on-partitions layout for every projection; DMA-transposing fp32 on device
is unsupported), weights are sliced/transposed per core.
"""

import numpy as np

import concourse.bacc as bacc
import concourse.mybir as mybir
import concourse.tile as tile
from concourse.bass import ds, ts
from concourse.bass_utils import run_bass_kernel_spmd

P = 128
S = 4096
D = 1024
H = 16
HD = 64
NC = 8
WQ = 512            # query-window width
NW = S // WQ        # 8 windows
KTW = WQ // P       # 4 k-tiles per window width
IO = D // P         # 8 contraction tiles for projections
G = 2               # k-tiles per exp group (2 psum banks)
SCALE = 1.0 / np.sqrt(HD)
NEG = -1.0e9

F32 = mybir.dt.float32
F32R = mybir.dt.float32r

_CACHED_NC = None


def _build():
    nc = bacc.Bacc("TRN2", target_bir_lowering=False, debug=False, num_devices=NC)

    xT = nc.dram_tensor("xT", [D, S], F32R, kind="ExternalInput")
    wq = nc.dram_tensor("wq", [D, P], F32R, kind="ExternalInput")
    wk = nc.dram_tensor("wk", [D, P], F32R, kind="ExternalInput")
    wv = nc.dram_tensor("wv", [D, P], F32R, kind="ExternalInput")
    wo = nc.dram_tensor("wo", [P, D], F32R, kind="ExternalInput")
    tri = nc.dram_tensor("tri", [P, P], F32, kind="ExternalInput")
    ident = nc.dram_tensor("ident", [P, P], F32R, kind="ExternalInput")
    y = nc.dram_tensor("y", [S, D], F32, kind="ExternalOutput")

    with tile.TileContext(nc) as tc:
        with (
            tc.tile_pool(name="const", bufs=1) as cpool,
            tc.tile_pool(name="bigs", bufs=1) as bigs,
            tc.tile_pool(name="xp", bufs=2) as xpool,
            tc.tile_pool(name="ptp", bufs=4) as ptpool,
            tc.tile_pool(name="ctxp", bufs=2) as ctxpool,
            tc.tile_pool(name="miscp", bufs=2) as miscp,
            tc.tile_pool(name="yp", bufs=2) as ypool,
            tc.tile_pool(name="ps_st", bufs=3, space="PSUM") as ps_st,
            tc.tile_pool(name="ps_ctx", bufs=2, space="PSUM") as ps_ctx,
        ):
            # ---- constants / weights to SBUF ----
            wq_sb = cpool.tile([P, IO, P], F32R, tag="wq")
            wk_sb = cpool.tile([P, IO, P], F32R, tag="wk")
            wv_sb = cpool.tile([P, IO, P], F32R, tag="wv")
            nc.sync.dma_start(wq_sb[:], wq.ap().rearrange("(o p) m -> p o m", p=P))
            nc.sync.dma_start(wk_sb[:], wk.ap().rearrange("(o p) m -> p o m", p=P))
            nc.sync.dma_start(wv_sb[:], wv.ap().rearrange("(o p) m -> p o m", p=P))
            # out-proj weights, both head-halves at partitions 0:64
            wo0 = cpool.tile([HD, D], F32R, tag="wo0")
            wo1 = cpool.tile([HD, D], F32R, tag="wo1")
            nc.sync.dma_start(wo0[:], wo.ap()[0:HD, :])
            nc.sync.dma_start(wo1[:], wo.ap()[HD:P, :])
            tri_sb = cpool.tile([P, P], F32, tag="tri")
            nc.sync.dma_start(tri_sb[:], tri.ap())
            id_sb = cpool.tile([P, P], F32R, tag="ident")
            nc.sync.dma_start(id_sb[:], ident.ap())
            ones_f32 = cpool.tile([P, HD], F32, tag="ones_f32")
            nc.vector.memset(ones_f32[:], 1.0)
            ones_sb = cpool.tile([P, HD], F32R, tag="ones")
            nc.vector.tensor_copy(ones_sb[:], ones_f32[:])

            QT = bigs.tile([P, S], F32R, tag="QT")
            KT = bigs.tile([P, S], F32R, tag="KT")
            VT = bigs.tile([P, S], F32R, tag="VT")
            # V in [k, d] layout + ones column at 64 (denominator row source)
            V65 = bigs.tile([P, 2, S // P, HD + 1], F32R, tag="V65")

            # ---- phase 1: QKV projections ----
            for c8 in range(NW):
                xt = xpool.tile([P, IO, WQ], F32R, tag="xt")
                nc.sync.dma_start(
                    xt[:],
                    xT.ap()[:, ds(c8 * WQ, WQ)].rearrange("(o p) s -> p o s", p=P),
                )
                for wsb, dest in ((wq_sb, QT), (wk_sb, KT), (wv_sb, VT)):
                    ps = ps_st.tile([P, G, WQ], F32, tag="st")
                    for io in range(IO):
                        nc.tensor.matmul(
                            ps[:, 0, :], wsb[:, io, :], xt[:, io, :],
                            start=(io == 0), stop=(io == IO - 1),
                        )
                    nc.vector.tensor_copy(dest[:, ds(c8 * WQ, WQ)], ps[:, 0, :])

            # ---- phase 2: V transpose ([d, s] -> [s, d] tiles) ----
            for t in range(S // P):
                tp = ps_st.tile([P, G, WQ], F32, tag="st")
                tpr = tp[:, 0, 0:P].bitcast(F32R)
                nc.tensor.transpose(tpr, VT[:, ds(t * P, P)], id_sb[:])
                nc.vector.tensor_copy(V65[:, 0, t, 0:HD], tpr[:, 0:HD])
                nc.vector.tensor_copy(V65[:, 1, t, 0:HD], tpr[:, HD:P])
            for h in (0, 1):
                nc.vector.tensor_copy(V65[:, h, :, HD], ones_f32[:, 0 : S // P])

            # ---- phase 3: attention + out-projection, one q-window at a time ----
            for w in range(NW):
                qsl = ds(w * WQ, WQ)
                nkt = KTW * (w + 1)          # causal k-tiles for this window
                ctx_a = ps_ctx.tile([P, WQ], F32, tag="ctx")
                ctx_b = ps_ctx.tile([P, WQ], F32, tag="ctx")
                ctxs = [ctx_a, ctx_b]

                for g0 in range(0, nkt, G):
                    kts = list(range(g0, min(g0 + G, nkt)))
                    gn = len(kts)
                    sts = []
                    for h in (0, 1):
                        ph = ds(HD * h, HD)
                        st = ps_st.tile([P, G, WQ], F32, tag="st")
                        sts.append(st)
                        for j, kt in enumerate(kts):
                            nc.tensor.matmul(
                                st[:, j, :],
                                KT[ph, ds(kt * P, P)], QT[ph, qsl],
                                start=True, stop=True,
                                tile_position=(HD * h, 0),
                            )
                            jo = kt - KTW * w
                            if jo >= 0:  # diagonal tile: causal mask
                                if jo > 0:
                                    nc.vector.memset(st[:, j, 0 : P * jo], NEG)
                                nc.vector.tensor_add(
                                    st[:, j, ds(P * jo, P)],
                                    st[:, j, ds(P * jo, P)], tri_sb[:],
                                )
                    pts = []
                    for h in (0, 1):
                        pt = ptpool.tile([P, G, WQ], F32R, tag="pt")
                        pts.append(pt)
                        nc.scalar.activation(
                            pt[:, 0:gn, :], sts[h][:, 0:gn, :],
                            mybir.ActivationFunctionType.Exp, scale=SCALE,
                        )
                    for j, kt in enumerate(kts):
                        for h in (0, 1):
                            # ctx (+ denominator row 64): V65.T @ PT
                            nc.tensor.matmul(
                                ctxs[h][0 : HD + 1, :],
                                V65[:, h, kt, :], pts[h][:, j, :],
                                start=(kt == 0), stop=(kt == nkt - 1),
                            )

                # normalize: ctxn_h[d, q] = ctx_h[d, q] * (1/den_h[q])
                recip = miscp.tile([P, 2, WQ], F32R, tag="recip")
                with nc.allow_low_precision(reason="fp32r has fp32 width"):
                    for h in (0, 1):
                        nc.vector.reciprocal(
                            recip[HD : HD + 1, h, :], ctxs[h][HD : HD + 1, :]
                        )
                ctxns = []
                for h in (0, 1):
                    bc = ps_st.tile([P, G, WQ], F32, tag="st")
                    nc.tensor.matmul(
                        bc[0:HD, 0, :],
                        ones_sb[HD : HD + 1, 0:HD], recip[HD : HD + 1, h, :],
                        start=True, stop=True,
                        tile_position=(HD, 0),
                    )
                    bcs = miscp.tile([HD, WQ], F32, tag=f"bcs{h}")
                    nc.vector.tensor_copy(bcs[:], bc[0:HD, 0, :])
                    ctxn = ctxpool.tile([HD, WQ], F32R, tag=f"ctxn{h}")
                    ctxns.append(ctxn)
                    nc.vector.tensor_mul(ctxn[:], ctxs[h][0:HD, :], bcs[:])

                # out-projection for this window's 4 s-tiles
                for t4 in range(KTW):
                    ysb = ypool.tile([P, D], F32, tag="ysb")
                    for oc in range(2):
                        yps = ps_st.tile([P, G, WQ], F32, tag="st")
                        nc.tensor.matmul(
                            yps[:, 0, :],
                            ctxns[0][:, ds(t4 * P, P)], wo0[:, ds(oc * WQ, WQ)],
                            start=True, stop=False,
                        )
                        nc.tensor.matmul(
                            yps[:, 0, :],
                            ctxns[1][:, ds(t4 * P, P)], wo1[:, ds(oc * WQ, WQ)],
                            start=False, stop=True,
                        )
                        nc.vector.tensor_copy(ysb[:, ds(oc * WQ, WQ)], yps[:, 0, :])
                    nc.sync.dma_start(y.ap()[ds(w * WQ + t4 * P, P), :], ysb[:])

    nc.compile()
    return nc


def _get_nc():
    global _CACHED_NC
    if _CACHED_NC is None:
        _CACHED_NC = _build()
    return _CACHED_NC


def kernel(x, Wq, Wk, Wv, Wo, bo):
    x = np.asarray(x, dtype=np.float32)
    Wq = np.asarray(Wq, dtype=np.float32)
    Wk = np.asarray(Wk, dtype=np.float32)
    Wv = np.asarray(Wv, dtype=np.float32)
    Wo = np.asarray(Wo, dtype=np.float32)
    bo = np.asarray(bo, dtype=np.float32)

    xT = np.ascontiguousarray(x.reshape(S, D).T)
    col = np.arange(P)
    tri = np.where(col[None, :] >= col[:, None], 0.0, NEG).astype(np.float32)
    ident = np.eye(P, dtype=np.float32)

    in_maps = []
    for c in range(NC):
        dsl = slice(P * c, P * (c + 1))
        in_maps.append({
            "xT": xT,
            "wq": np.ascontiguousarray(Wq[dsl, :].T),
            "wk": np.ascontiguousarray(Wk[dsl, :].T),
            "wv": np.ascontiguousarray(Wv[dsl, :].T),
            "wo": np.ascontiguousarray(Wo[:, dsl].T),
            "tri": tri,
            "ident": ident,
        })

    nc = _get_nc()
    res = run_bass_kernel_spmd(nc, in_maps, core_ids=list(range(NC)))
    out = np.zeros((S, D), dtype=np.float32)
    for c in range(NC):
        out += res.results[c]["y"]
    out += bo[None, :]
    return out.reshape(1, S, D)


# revision 6
# speedup vs baseline: 1.0727x; 1.0727x over previous
"""Causal multi-head attention (B=1, S=4096, D=1024, H=16, HD=64) on 8 TRN2
NeuronCores, sharded 2 heads per core (tensor parallel).

Per core c (heads 2c, 2c+1; d-slice [128c, 128c+128)):
  - QT/KT/VT [128, 4096] = W_slice @ x.T via fp32r matmuls (xT streamed).
  - V transposed back to [s, d] layout via PE identity-transpose, with a ones
    column appended (V65) so the attention matmul also produces the softmax
    denominator as output row 64.
  - Flash-style causal attention with transposed scores ST[k, q]:
    exp on ScalarE (no running max: scaled scores are in [-4.1, 4.1] for this
    problem family; exp is safe in fp32). Scores for the two heads run on
    disjoint PE row-groups (partitions 0:64 / 64:128). Causal masking via a
    small triangular additive mask on the diagonal 128x128 blocks plus
    column-trimmed context matmuls (invalid key columns never enter ctx).
  - Per-query normalization via a K=1 broadcast matmul + VectorE multiply.
  - Partial out-projection y_c = ctx_c @ Wo[:, d-slice].T -> [4096, 1024].
Host: y = sum_c y_c + bo.

The emission order pipelines phases: QKV projection for s-chunk w+2 and the
V-transpose for chunk w+1 are issued before attention window w, so projection
DMA/PE work hides under the (ScalarE-bound) attention of earlier windows.

Inputs are host-prepared: x is transposed once on host (the kernel needs
D-on-partitions layout for every projection; DMA-transposing fp32 on device
is unsupported), weights are sliced/transposed per core.
"""

import numpy as np

import concourse.bacc as bacc
import concourse.mybir as mybir
import concourse.tile as tile
from concourse.bass import ds, ts
from concourse.bass_utils import run_bass_kernel_spmd

P = 128
S = 4096
D = 1024
H = 16
HD = 64
NC = 8
WQ = 512            # query-window width
NW = S // WQ        # 8 windows
KTW = WQ // P       # 4 k-tiles per window width
IO = D // P         # 8 contraction tiles for projections
G = 3               # k-tiles per exp group (= banks per score tile)
SCALE = 1.0 / np.sqrt(HD)
NEG = -1.0e9

F32 = mybir.dt.float32
F32R = mybir.dt.float32r

_CACHED_NC = None


def _build():
    nc = bacc.Bacc("TRN2", target_bir_lowering=False, debug=False, num_devices=NC)

    xT = nc.dram_tensor("xT", [D, S], F32R, kind="ExternalInput")
    wq = nc.dram_tensor("wq", [D, P], F32R, kind="ExternalInput")
    wk = nc.dram_tensor("wk", [D, P], F32R, kind="ExternalInput")
    wv = nc.dram_tensor("wv", [D, P], F32R, kind="ExternalInput")
    wo = nc.dram_tensor("wo", [P, D], F32R, kind="ExternalInput")
    tri = nc.dram_tensor("tri", [P, P], F32, kind="ExternalInput")
    ident = nc.dram_tensor("ident", [P, P], F32R, kind="ExternalInput")
    y = nc.dram_tensor("y", [S, D], F32, kind="ExternalOutput")

    with tile.TileContext(nc) as tc:
        with (
            tc.tile_pool(name="const", bufs=1) as cpool,
            tc.tile_pool(name="bigs", bufs=1) as bigs,
            tc.tile_pool(name="xp", bufs=2) as xpool,
            tc.tile_pool(name="ptp", bufs=4) as ptpool,
            tc.tile_pool(name="ctxp", bufs=2) as ctxpool,
            tc.tile_pool(name="miscp", bufs=2) as miscp,
            tc.tile_pool(name="yp", bufs=3) as ypool,
            tc.tile_pool(name="ps_st", bufs=2, space="PSUM") as ps_st,
            tc.tile_pool(name="ps_ctx", bufs=2, space="PSUM") as ps_ctx,
        ):
            # ---- constants / weights to SBUF ----
            wq_sb = cpool.tile([P, IO, P], F32R, tag="wq")
            wk_sb = cpool.tile([P, IO, P], F32R, tag="wk")
            wv_sb = cpool.tile([P, IO, P], F32R, tag="wv")
            nc.sync.dma_start(wq_sb[:], wq.ap().rearrange("(o p) m -> p o m", p=P))
            nc.sync.dma_start(wk_sb[:], wk.ap().rearrange("(o p) m -> p o m", p=P))
            nc.sync.dma_start(wv_sb[:], wv.ap().rearrange("(o p) m -> p o m", p=P))
            # out-proj weights, both head-halves at partitions 0:64
            wo0 = cpool.tile([HD, D], F32R, tag="wo0")
            wo1 = cpool.tile([HD, D], F32R, tag="wo1")
            nc.sync.dma_start(wo0[:], wo.ap()[0:HD, :])
            nc.sync.dma_start(wo1[:], wo.ap()[HD:P, :])
            tri_sb = cpool.tile([P, P], F32, tag="tri")
            nc.sync.dma_start(tri_sb[:], tri.ap())
            id_sb = cpool.tile([P, P], F32R, tag="ident")
            nc.sync.dma_start(id_sb[:], ident.ap())
            ones_f32 = cpool.tile([P, HD], F32, tag="ones_f32")
            nc.vector.memset(ones_f32[:], 1.0)
            ones_sb = cpool.tile([P, HD], F32R, tag="ones")
            nc.vector.tensor_copy(ones_sb[:], ones_f32[:])

            QT = bigs.tile([P, S], F32R, tag="QT")
            KT = bigs.tile([P, S], F32R, tag="KT")
            VT = bigs.tile([P, S], F32R, tag="VT")
            # V in [k, d] layout + ones column at 64 (denominator row source)
            V65 = bigs.tile([P, 2, S // P, HD + 1], F32R, tag="V65")
            for h in (0, 1):
                nc.vector.tensor_copy(V65[:, h, :, HD], ones_f32[:, 0 : S // P])

            def qkv_chunk(c8):
                xt = xpool.tile([P, IO, WQ], F32R, tag="xt")
                nc.sync.dma_start(
                    xt[:],
                    xT.ap()[:, ds(c8 * WQ, WQ)].rearrange("(o p) s -> p o s", p=P),
                )
                for wsb, dest in ((wq_sb, QT), (wk_sb, KT), (wv_sb, VT)):
                    ps = ps_st.tile([P, G, WQ], F32, tag="st")
                    for io in range(IO):
                        nc.tensor.matmul(
                            ps[:, 0, :], wsb[:, io, :], xt[:, io, :],
                            start=(io == 0), stop=(io == IO - 1),
                        )
                    nc.any.tensor_copy(dest[:, ds(c8 * WQ, WQ)], ps[:, 0, :])

            def vtrans_chunk(c8):
                for t in range(KTW * c8, KTW * (c8 + 1)):
                    tp = ps_st.tile([P, G, WQ], F32, tag="st")
                    tpr = tp[:, 0, 0:P].bitcast(F32R)
                    nc.tensor.transpose(tpr, VT[:, ds(t * P, P)], id_sb[:])
                    nc.any.tensor_copy(V65[:, 0, t, 0:HD], tpr[:, 0:HD])
                    nc.any.tensor_copy(V65[:, 1, t, 0:HD], tpr[:, HD:P])

            def attention_window(w):
                qsl = ds(w * WQ, WQ)
                nkt = KTW * (w + 1)          # causal k-tiles for this window
                ctx_a = ps_ctx.tile([P, WQ], F32, tag="ctx")
                ctx_b = ps_ctx.tile([P, WQ], F32, tag="ctx")
                ctxs = [ctx_a, ctx_b]

                for g0 in range(0, nkt, G):
                    kts = list(range(g0, min(g0 + G, nkt)))
                    gn = len(kts)
                    sts = []
                    for h in (0, 1):
                        ph = ds(HD * h, HD)
                        st = ps_st.tile([P, G, WQ], F32, tag="st")
                        sts.append(st)
                        for j, kt in enumerate(kts):
                            nc.tensor.matmul(
                                st[:, j, :],
                                KT[ph, ds(kt * P, P)], QT[ph, qsl],
                                start=True, stop=True,
                                tile_position=(HD * h, 0),
                            )
                            jo = kt - KTW * w
                            if jo >= 0:  # diagonal tile: triangular causal mask
                                nc.vector.tensor_add(
                                    st[:, j, ds(P * jo, P)],
                                    st[:, j, ds(P * jo, P)], tri_sb[:],
                                )
                    pts = []
                    for h in (0, 1):
                        pt = ptpool.tile([P, G, WQ], F32R, tag="pt")
                        pts.append(pt)
                        nc.scalar.activation(
                            pt[:, 0:gn, :], sts[h][:, 0:gn, :],
                            mybir.ActivationFunctionType.Exp, scale=SCALE,
                        )
                    for j, kt in enumerate(kts):
                        jo = kt - KTW * w
                        # columns < 128*jo are fully masked: trim them out of
                        # the ctx matmul instead of zeroing PT
                        coff = P * jo if jo > 0 else 0
                        for h in (0, 1):
                            nc.tensor.matmul(
                                ctxs[h][0 : HD + 1, coff:WQ],
                                V65[:, h, kt, :], pts[h][:, j, coff:WQ],
                                start=(kt == 0), stop=(kt == nkt - 1),
                            )

                # normalize: ctxn_h[d, q] = ctx_h[d, q] * (1/den_h[q])
                recip = miscp.tile([P, 2, WQ], F32R, tag="recip")
                with nc.allow_low_precision(reason="fp32r has fp32 width"):
                    for h in (0, 1):
                        nc.vector.reciprocal(
                            recip[HD : HD + 1, h, :], ctxs[h][HD : HD + 1, :]
                        )
                ctxns = []
                for h in (0, 1):
                    bc = ps_st.tile([P, G, WQ], F32, tag="st")
                    nc.tensor.matmul(
                        bc[0:HD, 0, :],
                        ones_sb[HD : HD + 1, 0:HD], recip[HD : HD + 1, h, :],
                        start=True, stop=True,
                        tile_position=(HD, 0),
                    )
                    bcs = miscp.tile([HD, WQ], F32, tag=f"bcs{h}")
                    nc.any.tensor_copy(bcs[:], bc[0:HD, 0, :])
                    ctxn = ctxpool.tile([HD, WQ], F32R, tag=f"ctxn{h}")
                    ctxns.append(ctxn)
                    nc.vector.tensor_mul(ctxn[:], ctxs[h][0:HD, :], bcs[:])

                # out-projection for this window's 4 s-tiles
                for t4 in range(KTW):
                    ysb = ypool.tile([P, D], F32, tag="ysb")
                    for oc in range(2):
                        yps = ps_st.tile([P, G, WQ], F32, tag="st")
                        nc.tensor.matmul(
                            yps[:, 0, :],
                            ctxns[0][:, ds(t4 * P, P)], wo0[:, ds(oc * WQ, WQ)],
                            start=True, stop=False,
                        )
                        nc.tensor.matmul(
                            yps[:, 0, :],
                            ctxns[1][:, ds(t4 * P, P)], wo1[:, ds(oc * WQ, WQ)],
                            start=False, stop=True,
                        )
                        nc.any.tensor_copy(ysb[:, ds(oc * WQ, WQ)], yps[:, 0, :])
                    nc.sync.dma_start(y.ap()[ds(w * WQ + t4 * P, P), :], ysb[:])

            # ---- software-pipelined emission ----
            qkv_chunk(0)
            qkv_chunk(1)
            vtrans_chunk(0)
            for w in range(NW):
                if w + 2 < NW:
                    qkv_chunk(w + 2)
                if w + 1 < NW:
                    vtrans_chunk(w + 1)
                attention_window(w)

    nc.compile()
    return nc


def _get_nc():
    global _CACHED_NC
    if _CACHED_NC is None:
        _CACHED_NC = _build()
    return _CACHED_NC


def kernel(x, Wq, Wk, Wv, Wo, bo):
    x = np.asarray(x, dtype=np.float32)
    Wq = np.asarray(Wq, dtype=np.float32)
    Wk = np.asarray(Wk, dtype=np.float32)
    Wv = np.asarray(Wv, dtype=np.float32)
    Wo = np.asarray(Wo, dtype=np.float32)
    bo = np.asarray(bo, dtype=np.float32)

    xT = np.ascontiguousarray(x.reshape(S, D).T)
    col = np.arange(P)
    tri = np.where(col[None, :] >= col[:, None], 0.0, NEG).astype(np.float32)
    ident = np.eye(P, dtype=np.float32)

    in_maps = []
    for c in range(NC):
        dsl = slice(P * c, P * (c + 1))
        in_maps.append({
            "xT": xT,
            "wq": np.ascontiguousarray(Wq[dsl, :].T),
            "wk": np.ascontiguousarray(Wk[dsl, :].T),
            "wv": np.ascontiguousarray(Wv[dsl, :].T),
            "wo": np.ascontiguousarray(Wo[:, dsl].T),
            "tri": tri,
            "ident": ident,
        })

    nc = _get_nc()
    res = run_bass_kernel_spmd(nc, in_maps, core_ids=list(range(NC)))
    out = np.zeros((S, D), dtype=np.float32)
    for c in range(NC):
        out += res.results[c]["y"]
    out += bo[None, :]
    return out.reshape(1, S, D)


# revision 10
# speedup vs baseline: 1.3063x; 1.2177x over previous
"""Causal multi-head attention (B=1, S=4096, D=1024, H=16, HD=64) on 8 TRN2
NeuronCores, sharded 2 heads per core (tensor parallel).

Per core c (heads 2c, 2c+1; d-slice [128c, 128c+128)):
  - QT/KT/VT [128, 4096] = W_slice @ x.T via fp32r matmuls (xT streamed).
  - V transposed back to [s, d] layout via PE identity-transpose, with a ones
    column appended (V65) so the attention matmul also produces the softmax
    denominator as output row 64.
  - Flash-style causal attention with transposed scores ST[k, q]:
    exp on ScalarE (no running max: scaled scores are in [-4.1, 4.1] for this
    problem family; exp is safe in fp32). Scores for the two heads run on
    disjoint PE row-groups (partitions 0:64 / 64:128). Causal masking via a
    small triangular additive mask on the diagonal 128x128 blocks plus
    column-trimmed context matmuls (invalid key columns never enter ctx).
  - Per-query normalization via a K=1 broadcast matmul + VectorE multiply.
  - Partial out-projection y_c = ctx_c @ Wo[:, d-slice].T -> [4096, 1024].
Host: y = sum_c y_c + bo.

The emission order pipelines phases: QKV projection for s-chunk w+2 and the
V-transpose for chunk w+1 are issued before attention window w, so projection
DMA/PE work hides under the (ScalarE-bound) attention of earlier windows.

Inputs are host-prepared: x is transposed once on host (the kernel needs
D-on-partitions layout for every projection; DMA-transposing fp32 on device
is unsupported), weights are sliced/transposed per core.
"""

import numpy as np

import concourse.bacc as bacc
import concourse.mybir as mybir
import concourse.tile as tile
from concourse.bass import ds
from concourse.bass_utils import run_bass_kernel_spmd

P = 128
S = 4096
D = 1024
H = 16
HD = 64
NC = 8
WQ = 512            # query-window width
NW = S // WQ        # 8 windows
KTW = WQ // P       # 4 k-tiles per window width
IO = D // P         # 8 contraction tiles for projections
G = 2               # head-slots per score tile (one bank per head)
SCALE = 1.0 / np.sqrt(HD)
NEG = -1.0e9

F32 = mybir.dt.float32
F32R = mybir.dt.float32r

_CACHED_NC = None


def _build():
    nc = bacc.Bacc("TRN2", target_bir_lowering=False, debug=False, num_devices=NC)

    xT = nc.dram_tensor("xT", [D, S], F32R, kind="ExternalInput")
    wq = nc.dram_tensor("wq", [D, P], F32R, kind="ExternalInput")
    wk = nc.dram_tensor("wk", [D, P], F32R, kind="ExternalInput")
    wv = nc.dram_tensor("wv", [D, P], F32R, kind="ExternalInput")
    wo = nc.dram_tensor("wo", [P, D], F32R, kind="ExternalInput")
    tri = nc.dram_tensor("tri", [P, P], F32, kind="ExternalInput")
    ident = nc.dram_tensor("ident", [P, P], F32R, kind="ExternalInput")
    y = nc.dram_tensor("y", [S, D], F32, kind="ExternalOutput")

    with tile.TileContext(nc) as tc:
        with (
            tc.tile_pool(name="const", bufs=1) as cpool,
            tc.tile_pool(name="bigs", bufs=1) as bigs,
            tc.tile_pool(name="xp", bufs=2) as xpool,
            tc.tile_pool(name="ptp", bufs=6) as ptpool,
            tc.tile_pool(name="ctxp", bufs=2) as ctxpool,
            tc.tile_pool(name="miscp", bufs=2) as miscp,
            tc.tile_pool(name="yp", bufs=3) as ypool,
            tc.tile_pool(name="ps_st", bufs=3, space="PSUM") as ps_st,
            tc.tile_pool(name="ps_ctx", bufs=2, space="PSUM") as ps_ctx,
        ):
            # ---- constants / weights to SBUF ----
            wq_sb = cpool.tile([P, IO, P], F32R, tag="wq")
            wk_sb = cpool.tile([P, IO, P], F32R, tag="wk")
            wv_sb = cpool.tile([P, IO, P], F32R, tag="wv")
            nc.sync.dma_start(wq_sb[:], wq.ap().rearrange("(o p) m -> p o m", p=P))
            nc.sync.dma_start(wk_sb[:], wk.ap().rearrange("(o p) m -> p o m", p=P))
            nc.sync.dma_start(wv_sb[:], wv.ap().rearrange("(o p) m -> p o m", p=P))
            # out-proj weights, both head-halves at partitions 0:64
            wo0 = cpool.tile([HD, D], F32R, tag="wo0")
            wo1 = cpool.tile([HD, D], F32R, tag="wo1")
            nc.sync.dma_start(wo0[:], wo.ap()[0:HD, :])
            nc.sync.dma_start(wo1[:], wo.ap()[HD:P, :])
            tri_sb = cpool.tile([P, P], F32, tag="tri")
            nc.sync.dma_start(tri_sb[:], tri.ap())
            id_sb = cpool.tile([P, P], F32R, tag="ident")
            nc.sync.dma_start(id_sb[:], ident.ap())
            ones_f32 = cpool.tile([P, HD], F32, tag="ones_f32")
            nc.vector.memset(ones_f32[:], 1.0)
            ones_sb = cpool.tile([P, HD], F32R, tag="ones")
            nc.vector.tensor_copy(ones_sb[:], ones_f32[:])

            QT = bigs.tile([P, S], F32R, tag="QT")
            KT = bigs.tile([P, S], F32R, tag="KT")
            VT = bigs.tile([P, S], F32R, tag="VT")
            # V in [k, d] layout + ones column at 64 (denominator row source)
            V65 = bigs.tile([P, 2, S // P, HD + 1], F32R, tag="V65")
            for h in (0, 1):
                nc.vector.tensor_copy(V65[:, h, :, HD], ones_f32[:, 0 : S // P])

            def qkv_chunk(c8):
                xt = xpool.tile([P, IO, WQ], F32R, tag="xt")
                nc.sync.dma_start(
                    xt[:],
                    xT.ap()[:, ds(c8 * WQ, WQ)].rearrange("(o p) s -> p o s", p=P),
                )
                for wsb, dest in ((wq_sb, QT), (wk_sb, KT), (wv_sb, VT)):
                    ps = ps_st.tile([P, G, WQ], F32, tag="st")
                    for io in range(IO):
                        nc.tensor.matmul(
                            ps[:, 0, :], wsb[:, io, :], xt[:, io, :],
                            start=(io == 0), stop=(io == IO - 1),
                        )
                    nc.vector.tensor_copy(dest[:, ds(c8 * WQ, WQ)], ps[:, 0, :])

            def vtrans_chunk(c8):
                for t in range(KTW * c8, KTW * (c8 + 1)):
                    tp = ps_st.tile([P, G, WQ], F32, tag="st")
                    tpr = tp[:, 0, 0:P].bitcast(F32R)
                    nc.tensor.transpose(tpr, VT[:, ds(t * P, P)], id_sb[:])
                    nc.vector.tensor_copy(V65[:, 0, t, 0:HD], tpr[:, 0:HD])
                    nc.vector.tensor_copy(V65[:, 1, t, 0:HD], tpr[:, HD:P])

            def attention_window(w):
                qsl = ds(w * WQ, WQ)
                nkt = KTW * (w + 1)          # causal k-tiles for this window
                ctx_a = ps_ctx.tile([P, WQ], F32, tag="ctx")
                ctx_b = ps_ctx.tile([P, WQ], F32, tag="ctx")
                ctxs = [ctx_a, ctx_b]

                for kt in range(nkt):
                    jo = kt - KTW * w
                    st = ps_st.tile([P, G, WQ], F32, tag="st")
                    for h in (0, 1):
                        ph = ds(HD * h, HD)
                        nc.tensor.matmul(
                            st[:, h, :],
                            KT[ph, ds(kt * P, P)], QT[ph, qsl],
                            start=True, stop=True,
                            tile_position=(HD * h, 0),
                        )
                        if jo >= 0:  # diagonal tile: triangular causal mask
                            nc.vector.tensor_add(
                                st[:, h, ds(P * jo, P)],
                                st[:, h, ds(P * jo, P)], tri_sb[:],
                            )
                    pt = ptpool.tile([P, G, WQ], F32R, tag="pt")
                    nc.scalar.activation(
                        pt[:], st[:],
                        mybir.ActivationFunctionType.Exp, scale=SCALE,
                    )
                    # columns < 128*jo are fully masked: trim them out of
                    # the ctx matmul instead of zeroing PT
                    coff = P * jo if jo > 0 else 0
                    for h in (0, 1):
                        nc.tensor.matmul(
                            ctxs[h][0 : HD + 1, coff:WQ],
                            V65[:, h, kt, :], pt[:, h, coff:WQ],
                            start=(kt == 0), stop=(kt == nkt - 1),
                        )

                # normalize: ctxn_h[d, q] = ctx_h[d, q] * (1/den_h[q])
                recip = miscp.tile([P, 2, WQ], F32R, tag="recip")
                with nc.allow_low_precision(reason="fp32r has fp32 width"):
                    for h in (0, 1):
                        nc.vector.reciprocal(
                            recip[HD : HD + 1, h, :], ctxs[h][HD : HD + 1, :]
                        )
                ctxns = []
                for h in (0, 1):
                    bc = ps_st.tile([P, G, WQ], F32, tag="st")
                    nc.tensor.matmul(
                        bc[0:HD, 0, :],
                        ones_sb[HD : HD + 1, 0:HD], recip[HD : HD + 1, h, :],
                        start=True, stop=True,
                        tile_position=(HD, 0),
                    )
                    bcs = miscp.tile([HD, WQ], F32, tag=f"bcs{h}")
                    nc.vector.tensor_copy(bcs[:], bc[0:HD, 0, :])
                    ctxn = ctxpool.tile([HD, WQ], F32R, tag=f"ctxn{h}")
                    ctxns.append(ctxn)
                    nc.vector.tensor_mul(ctxn[:], ctxs[h][0:HD, :], bcs[:])

                # out-projection for this window's 4 s-tiles
                for t4 in range(KTW):
                    ysb = ypool.tile([P, D], F32, tag="ysb")
                    for oc in range(2):
                        yps = ps_st.tile([P, G, WQ], F32, tag="st")
                        nc.tensor.matmul(
                            yps[:, 0, :],
                            ctxns[0][:, ds(t4 * P, P)], wo0[:, ds(oc * WQ, WQ)],
                            start=True, stop=False,
                        )
                        nc.tensor.matmul(
                            yps[:, 0, :],
                            ctxns[1][:, ds(t4 * P, P)], wo1[:, ds(oc * WQ, WQ)],
                            start=False, stop=True,
                        )
                        nc.vector.tensor_copy(ysb[:, ds(oc * WQ, WQ)], yps[:, 0, :])
                    nc.sync.dma_start(y.ap()[ds(w * WQ + t4 * P, P), :], ysb[:])

            # ---- software-pipelined emission ----
            qkv_chunk(0)
            qkv_chunk(1)
            vtrans_chunk(0)
            for w in range(NW):
                if w + 2 < NW:
                    qkv_chunk(w + 2)
                if w + 1 < NW:
                    vtrans_chunk(w + 1)
                attention_window(w)

    nc.compile()
    return nc


def _get_nc():
    global _CACHED_NC
    if _CACHED_NC is None:
        _CACHED_NC = _build()
    return _CACHED_NC


def kernel(x, Wq, Wk, Wv, Wo, bo):
    x = np.asarray(x, dtype=np.float32)
    Wq = np.asarray(Wq, dtype=np.float32)
    Wk = np.asarray(Wk, dtype=np.float32)
    Wv = np.asarray(Wv, dtype=np.float32)
    Wo = np.asarray(Wo, dtype=np.float32)
    bo = np.asarray(bo, dtype=np.float32)

    xT = np.ascontiguousarray(x.reshape(S, D).T)
    col = np.arange(P)
    tri = np.where(col[None, :] >= col[:, None], 0.0, NEG).astype(np.float32)
    ident = np.eye(P, dtype=np.float32)

    in_maps = []
    for c in range(NC):
        dsl = slice(P * c, P * (c + 1))
        in_maps.append({
            "xT": xT,
            "wq": np.ascontiguousarray(Wq[dsl, :].T),
            "wk": np.ascontiguousarray(Wk[dsl, :].T),
            "wv": np.ascontiguousarray(Wv[dsl, :].T),
            "wo": np.ascontiguousarray(Wo[:, dsl].T),
            "tri": tri,
            "ident": ident,
        })

    nc = _get_nc()
    res = run_bass_kernel_spmd(nc, in_maps, core_ids=list(range(NC)))
    out = np.zeros((S, D), dtype=np.float32)
    for c in range(NC):
        out += res.results[c]["y"]
    out += bo[None, :]
    return out.reshape(1, S, D)


# revision 19
# speedup vs baseline: 1.4507x; 1.1105x over previous
"""Causal multi-head attention (B=1, S=4096, D=1024, H=16, HD=64) on 8 TRN2
NeuronCores, sharded 2 heads per core (tensor parallel).

Per core c (heads 2c, 2c+1; d-slice [128c, 128c+128)):
  - QT/KT/VT [128, 4096] = W_slice @ x.T via fp32r matmuls (xT streamed).
  - V transposed back to [s, d] layout via PE identity-transpose, with a ones
    column appended (V65) so the attention matmul also produces the softmax
    denominator as output row 64.
  - Flash-style causal attention with transposed scores ST[k, q]:
    exp on ScalarE (no running max: scaled scores are in [-4.1, 4.1] for this
    problem family; exp is safe in fp32). Scores for the two heads run on
    disjoint PE row-groups (partitions 0:64 / 64:128). Causal masking via a
    small triangular additive mask on the diagonal 128x128 blocks plus
    column-trimmed context matmuls (invalid key columns never enter ctx).
  - Per-query normalization via a K=1 broadcast matmul + VectorE multiply.
  - Partial out-projection y_c = ctx_c @ Wo[:, d-slice].T -> [4096, 1024].
Host: y = sum_c y_c + bo.

The emission order pipelines phases: QKV projection for s-chunk w+2 and the
V-transpose for chunk w+1 are issued before attention window w, so projection
DMA/PE work hides under the (ScalarE-bound) attention of earlier windows.

Inputs are host-prepared: x is transposed once on host (the kernel needs
D-on-partitions layout for every projection; DMA-transposing fp32 on device
is unsupported), weights are sliced/transposed per core.
"""

import numpy as np

import concourse.bacc as bacc
import concourse.mybir as mybir
import concourse.tile as tile
from concourse.bass import ds
from concourse.bass_utils import run_bass_kernel_spmd

P = 128
S = 4096
D = 1024
H = 16
HD = 64
NC = 8
WQ = 512            # query-window width
NW = S // WQ        # 8 windows
KTW = WQ // P       # 4 k-tiles per window width
IO = D // P         # 8 contraction tiles for projections
G = 2               # head-slots per score tile (one bank per head)
SCALE = 1.0 / np.sqrt(HD)
NEG = -1.0e9

F32 = mybir.dt.float32
F32R = mybir.dt.float32r

_CACHED_NC = None


def _build():
    nc = bacc.Bacc("TRN2", target_bir_lowering=False, debug=False, num_devices=NC)

    xT = nc.dram_tensor("xT", [D, S], F32R, kind="ExternalInput")
    wq = nc.dram_tensor("wq", [D, P], F32R, kind="ExternalInput")
    wk = nc.dram_tensor("wk", [D, P], F32R, kind="ExternalInput")
    wv = nc.dram_tensor("wv", [D, P], F32R, kind="ExternalInput")
    wo = nc.dram_tensor("wo", [P, D], F32R, kind="ExternalInput")
    tri = nc.dram_tensor("tri", [P, P], F32, kind="ExternalInput")
    ident = nc.dram_tensor("ident", [P, P], F32R, kind="ExternalInput")
    y = nc.dram_tensor("y", [S, D], F32, kind="ExternalOutput")

    with tile.TileContext(nc) as tc:
        with (
            tc.tile_pool(name="const", bufs=1) as cpool,
            tc.tile_pool(name="bigs", bufs=1) as bigs,
            tc.tile_pool(name="xp", bufs=2) as xpool,
            tc.tile_pool(name="ptp", bufs=10) as ptpool,
            tc.tile_pool(name="ctxp", bufs=2) as ctxpool,
            tc.tile_pool(name="miscp", bufs=2) as miscp,
            tc.tile_pool(name="yp", bufs=3) as ypool,
            tc.tile_pool(name="ps_st", bufs=3, space="PSUM") as ps_st,
            tc.tile_pool(name="ps_ctx", bufs=2, space="PSUM") as ps_ctx,
        ):
            # ---- constants / weights to SBUF ----
            wq_sb = cpool.tile([P, IO, P], F32R, tag="wq")
            wk_sb = cpool.tile([P, IO, P], F32R, tag="wk")
            wv_sb = cpool.tile([P, IO, P], F32R, tag="wv")
            nc.sync.dma_start(wq_sb[:], wq.ap().rearrange("(o p) m -> p o m", p=P))
            nc.sync.dma_start(wk_sb[:], wk.ap().rearrange("(o p) m -> p o m", p=P))
            nc.sync.dma_start(wv_sb[:], wv.ap().rearrange("(o p) m -> p o m", p=P))
            # out-proj weights, both head-halves at partitions 0:64
            wo0 = cpool.tile([HD, D], F32R, tag="wo0")
            wo1 = cpool.tile([HD, D], F32R, tag="wo1")
            nc.sync.dma_start(wo0[:], wo.ap()[0:HD, :])
            nc.sync.dma_start(wo1[:], wo.ap()[HD:P, :])
            tri_sb = cpool.tile([P, P], F32, tag="tri")
            nc.sync.dma_start(tri_sb[:], tri.ap())
            id_sb = cpool.tile([P, P], F32R, tag="ident")
            nc.sync.dma_start(id_sb[:], ident.ap())
            ones_f32 = cpool.tile([P, HD], F32, tag="ones_f32")
            nc.vector.memset(ones_f32[:], 1.0)
            ones_sb = cpool.tile([P, HD], F32R, tag="ones")
            nc.vector.tensor_copy(ones_sb[:], ones_f32[:])

            QT = bigs.tile([P, S], F32R, tag="QT")
            KT = bigs.tile([P, S], F32R, tag="KT")
            VT = bigs.tile([P, S], F32R, tag="VT")
            # V in [k, d] layout + ones column at 64 (denominator row source)
            V65 = bigs.tile([P, 2, S // P, HD + 1], F32R, tag="V65")
            for h in (0, 1):
                nc.vector.tensor_copy(V65[:, h, :, HD], ones_f32[:, 0 : S // P])

            def qkv_chunk(c8):
                xt = xpool.tile([P, IO, WQ], F32R, tag="xt")
                nc.sync.dma_start(
                    xt[:],
                    xT.ap()[:, ds(c8 * WQ, WQ)].rearrange("(o p) s -> p o s", p=P),
                )
                for wsb, dest in ((wq_sb, QT), (wk_sb, KT), (wv_sb, VT)):
                    ps = ps_st.tile([P, G, WQ], F32, tag="st")
                    for io in range(IO):
                        nc.tensor.matmul(
                            ps[:, 0, :], wsb[:, io, :], xt[:, io, :],
                            start=(io == 0), stop=(io == IO - 1),
                        )
                    nc.vector.tensor_copy(dest[:, ds(c8 * WQ, WQ)], ps[:, 0, :])

            def vtrans_chunk(c8):
                for t in range(KTW * c8, KTW * (c8 + 1)):
                    tp = ps_st.tile([P, G, WQ], F32, tag="st")
                    tpr = tp[:, 0, 0:P].bitcast(F32R)
                    nc.tensor.transpose(tpr, VT[:, ds(t * P, P)], id_sb[:])
                    nc.vector.tensor_copy(V65[:, 0, t, 0:HD], tpr[:, 0:HD])
                    nc.vector.tensor_copy(V65[:, 1, t, 0:HD], tpr[:, HD:P])

            def attention_window(w):
                qsl = ds(w * WQ, WQ)
                nkt = KTW * (w + 1)          # causal k-tiles for this window
                ctx_a = ps_ctx.tile([P, WQ], F32, tag="ctx")
                ctx_b = ps_ctx.tile([P, WQ], F32, tag="ctx")
                ctxs = [ctx_a, ctx_b]

                for kt in range(nkt):
                    jo = kt - KTW * w
                    st = ps_st.tile([P, G, WQ], F32, tag="st")
                    for h in (0, 1):
                        ph = ds(HD * h, HD)
                        nc.tensor.matmul(
                            st[:, h, :],
                            KT[ph, ds(kt * P, P)], QT[ph, qsl],
                            start=True, stop=True,
                            tile_position=(HD * h, 0),
                        )
                        if jo >= 0:  # diagonal tile: triangular causal mask
                            nc.vector.tensor_add(
                                st[:, h, ds(P * jo, P)],
                                st[:, h, ds(P * jo, P)], tri_sb[:],
                            )
                    pt = ptpool.tile([P, G, WQ], F32R, tag="pt")
                    nc.scalar.activation(
                        pt[:], st[:],
                        mybir.ActivationFunctionType.Exp, scale=SCALE,
                    )
                    # columns < 128*jo are fully masked: trim them out of
                    # the ctx matmul instead of zeroing PT
                    coff = P * jo if jo > 0 else 0
                    for h in (0, 1):
                        nc.tensor.matmul(
                            ctxs[h][0 : HD + 1, coff:WQ],
                            V65[:, h, kt, :], pt[:, h, coff:WQ],
                            start=(kt == 0), stop=(kt == nkt - 1),
                        )

                # normalize: ctxn_h[d, q] = ctx_h[d, q] * (1/den_h[q])
                recip = miscp.tile([P, 2, WQ], F32R, tag="recip")
                with nc.allow_low_precision(reason="fp32r has fp32 width"):
                    for h in (0, 1):
                        nc.vector.reciprocal(
                            recip[HD : HD + 1, h, :], ctxs[h][HD : HD + 1, :]
                        )
                ctxns = []
                for h in (0, 1):
                    bc = ps_st.tile([P, G, WQ], F32, tag="st")
                    nc.tensor.matmul(
                        bc[0:HD, 0, :],
                        ones_sb[HD : HD + 1, 0:HD], recip[HD : HD + 1, h, :],
                        start=True, stop=True,
                        tile_position=(HD, 0),
                    )
                    bcs = miscp.tile([HD, WQ], F32, tag=f"bcs{h}")
                    nc.vector.tensor_copy(bcs[:], bc[0:HD, 0, :])
                    ctxn = ctxpool.tile([HD, WQ], F32R, tag=f"ctxn{h}")
                    ctxns.append(ctxn)
                    nc.vector.tensor_mul(ctxn[:], ctxs[h][0:HD, :], bcs[:])

                # out-projection for this window's 4 s-tiles
                for t4 in range(KTW):
                    ysb = ypool.tile([P, D], F32, tag="ysb")
                    for oc in range(2):
                        yps = ps_st.tile([P, G, WQ], F32, tag="st")
                        nc.tensor.matmul(
                            yps[:, 0, :],
                            ctxns[0][:, ds(t4 * P, P)], wo0[:, ds(oc * WQ, WQ)],
                            start=True, stop=False,
                        )
                        nc.tensor.matmul(
                            yps[:, 0, :],
                            ctxns[1][:, ds(t4 * P, P)], wo1[:, ds(oc * WQ, WQ)],
                            start=False, stop=True,
                        )
                        nc.vector.tensor_copy(ysb[:, ds(oc * WQ, WQ)], yps[:, 0, :])
                    nc.sync.dma_start(y.ap()[ds(w * WQ + t4 * P, P), :], ysb[:])

            # ---- software-pipelined emission ----
            qkv_chunk(0)
            qkv_chunk(1)
            vtrans_chunk(0)
            for w in range(NW):
                if w + 2 < NW:
                    qkv_chunk(w + 2)
                if w + 1 < NW:
                    vtrans_chunk(w + 1)
                attention_window(w)

    nc.compile()
    return nc


def _get_nc():
    global _CACHED_NC
    if _CACHED_NC is None:
        _CACHED_NC = _build()
    return _CACHED_NC


def kernel(x, Wq, Wk, Wv, Wo, bo):
    x = np.asarray(x, dtype=np.float32)
    Wq = np.asarray(Wq, dtype=np.float32)
    Wk = np.asarray(Wk, dtype=np.float32)
    Wv = np.asarray(Wv, dtype=np.float32)
    Wo = np.asarray(Wo, dtype=np.float32)
    bo = np.asarray(bo, dtype=np.float32)

    xT = np.ascontiguousarray(x.reshape(S, D).T)
    col = np.arange(P)
    tri = np.where(col[None, :] >= col[:, None], 0.0, NEG).astype(np.float32)
    ident = np.eye(P, dtype=np.float32)

    in_maps = []
    for c in range(NC):
        dsl = slice(P * c, P * (c + 1))
        in_maps.append({
            "xT": xT,
            "wq": np.ascontiguousarray(Wq[dsl, :].T),
            "wk": np.ascontiguousarray(Wk[dsl, :].T),
            "wv": np.ascontiguousarray(Wv[dsl, :].T),
            "wo": np.ascontiguousarray(Wo[:, dsl].T),
            "tri": tri,
            "ident": ident,
        })

    nc = _get_nc()
    res = run_bass_kernel_spmd(nc, in_maps, core_ids=list(range(NC)))
    out = np.zeros((S, D), dtype=np.float32)
    for c in range(NC):
        out += res.results[c]["y"]
    out += bo[None, :]
    return out.reshape(1, S, D)


# revision 25
# speedup vs baseline: 1.4742x; 1.0162x over previous
"""Causal multi-head attention (B=1, S=4096, D=1024, H=16, HD=64) on 8 TRN2
NeuronCores, sharded 2 heads per core (tensor parallel).

Per core c (heads 2c, 2c+1; d-slice [128c, 128c+128)):
  - QT/KT/VT [128, 4096] = W_slice @ x.T via fp32r matmuls (xT streamed).
  - V transposed back to [s, d] layout via PE identity-transpose, with a ones
    column appended (V65) so the attention matmul also produces the softmax
    denominator as output row 64.
  - Flash-style causal attention with transposed scores ST[k, q]:
    exp on ScalarE (no running max: scaled scores are in [-4.1, 4.1] for this
    problem family; exp is safe in fp32). Scores for the two heads run on
    disjoint PE row-groups (partitions 0:64 / 64:128). Causal masking via a
    small triangular additive mask on the diagonal 128x128 blocks plus
    column-trimmed context matmuls (invalid key columns never enter ctx).
  - Per-query normalization via a K=1 broadcast matmul + VectorE multiply.
  - Partial out-projection y_c = ctx_c @ Wo[:, d-slice].T -> [4096, 1024].
Host: y = sum_c y_c + bo.

The emission order pipelines phases: QKV projection for s-chunk w+2 and the
V-transpose for chunk w+1 are issued before attention window w, so projection
DMA/PE work hides under the (ScalarE-bound) attention of earlier windows.

Inputs are host-prepared: x is transposed once on host (the kernel needs
D-on-partitions layout for every projection; DMA-transposing fp32 on device
is unsupported), weights are sliced/transposed per core.
"""

import numpy as np

import concourse.bacc as bacc
import concourse.mybir as mybir
import concourse.tile as tile
from concourse.bass import ds
from concourse.bass_utils import run_bass_kernel_spmd

P = 128
S = 4096
D = 1024
H = 16
HD = 64
NC = 8
WQ = 512            # query-window width
NW = S // WQ        # 8 windows
KTW = WQ // P       # 4 k-tiles per window width
IO = D // P         # 8 contraction tiles for projections
G = 2               # head-slots per score tile (one bank per head)
SCALE = 1.0 / np.sqrt(HD)
NEG = -1.0e9

F32 = mybir.dt.float32
F32R = mybir.dt.float32r

_CACHED_NC = None


def _build():
    nc = bacc.Bacc("TRN2", target_bir_lowering=False, debug=False, num_devices=NC)

    xT = nc.dram_tensor("xT", [D, S], F32R, kind="ExternalInput")
    wq = nc.dram_tensor("wq", [D, P], F32R, kind="ExternalInput")
    wk = nc.dram_tensor("wk", [D, P], F32R, kind="ExternalInput")
    wv = nc.dram_tensor("wv", [D, P], F32R, kind="ExternalInput")
    wo = nc.dram_tensor("wo", [P, D], F32R, kind="ExternalInput")
    tri = nc.dram_tensor("tri", [P, P], F32, kind="ExternalInput")
    ident = nc.dram_tensor("ident", [P, P], F32R, kind="ExternalInput")
    y = nc.dram_tensor("y", [S, D], F32, kind="ExternalOutput")

    with tile.TileContext(nc) as tc:
        with (
            tc.tile_pool(name="const", bufs=1) as cpool,
            tc.tile_pool(name="bigs", bufs=1) as bigs,
            tc.tile_pool(name="xp", bufs=2) as xpool,
            tc.tile_pool(name="ptp", bufs=8) as ptpool,
            tc.tile_pool(name="ctxp", bufs=2) as ctxpool,
            tc.tile_pool(name="miscp", bufs=2) as miscp,
            tc.tile_pool(name="yp", bufs=6) as ypool,
            tc.tile_pool(name="ps_st", bufs=3, space="PSUM") as ps_st,
            tc.tile_pool(name="ps_ctx", bufs=2, space="PSUM") as ps_ctx,
        ):
            # ---- constants / weights to SBUF ----
            wq_sb = cpool.tile([P, IO, P], F32R, tag="wq")
            wk_sb = cpool.tile([P, IO, P], F32R, tag="wk")
            wv_sb = cpool.tile([P, IO, P], F32R, tag="wv")
            nc.sync.dma_start(wq_sb[:], wq.ap().rearrange("(o p) m -> p o m", p=P))
            nc.sync.dma_start(wk_sb[:], wk.ap().rearrange("(o p) m -> p o m", p=P))
            nc.sync.dma_start(wv_sb[:], wv.ap().rearrange("(o p) m -> p o m", p=P))
            # out-proj weights, both head-halves at partitions 0:64
            wo0 = cpool.tile([HD, D], F32R, tag="wo0")
            wo1 = cpool.tile([HD, D], F32R, tag="wo1")
            nc.sync.dma_start(wo0[:], wo.ap()[0:HD, :])
            nc.sync.dma_start(wo1[:], wo.ap()[HD:P, :])
            tri_sb = cpool.tile([P, P], F32, tag="tri")
            nc.sync.dma_start(tri_sb[:], tri.ap())
            id_sb = cpool.tile([P, P], F32R, tag="ident")
            nc.sync.dma_start(id_sb[:], ident.ap())
            ones_f32 = cpool.tile([P, HD], F32, tag="ones_f32")
            nc.vector.memset(ones_f32[:], 1.0)
            ones_sb = cpool.tile([P, HD], F32R, tag="ones")
            nc.vector.tensor_copy(ones_sb[:], ones_f32[:])

            QT = bigs.tile([P, S], F32R, tag="QT")
            KT = bigs.tile([P, S], F32R, tag="KT")
            VT = bigs.tile([P, S], F32R, tag="VT")
            # V in [k, d] layout + ones column at 64 (denominator row source)
            V65 = bigs.tile([P, 2, S // P, HD + 1], F32R, tag="V65")
            for h in (0, 1):
                nc.vector.tensor_copy(V65[:, h, :, HD], ones_f32[:, 0 : S // P])

            def qkv_chunk(c8):
                xt = xpool.tile([P, IO, WQ], F32R, tag="xt")
                nc.sync.dma_start(
                    xt[:],
                    xT.ap()[:, ds(c8 * WQ, WQ)].rearrange("(o p) s -> p o s", p=P),
                )
                for wsb, dest in ((wq_sb, QT), (wk_sb, KT), (wv_sb, VT)):
                    ps = ps_st.tile([P, G, WQ], F32, tag="st")
                    for io in range(IO):
                        nc.tensor.matmul(
                            ps[:, 0, :], wsb[:, io, :], xt[:, io, :],
                            start=(io == 0), stop=(io == IO - 1),
                        )
                    nc.vector.tensor_copy(dest[:, ds(c8 * WQ, WQ)], ps[:, 0, :])

            def vtrans_chunk(c8):
                for t in range(KTW * c8, KTW * (c8 + 1)):
                    tp = ps_st.tile([P, G, WQ], F32, tag="st")
                    tpr = tp[:, 0, 0:P].bitcast(F32R)
                    nc.tensor.transpose(tpr, VT[:, ds(t * P, P)], id_sb[:])
                    nc.vector.tensor_copy(V65[:, 0, t, 0:HD], tpr[:, 0:HD])
                    nc.vector.tensor_copy(V65[:, 1, t, 0:HD], tpr[:, HD:P])

            def attention_window(w):
                qsl = ds(w * WQ, WQ)
                nkt = KTW * (w + 1)          # causal k-tiles for this window
                ctx_a = ps_ctx.tile([P, WQ], F32, tag="ctx")
                ctx_b = ps_ctx.tile([P, WQ], F32, tag="ctx")
                ctxs = [ctx_a, ctx_b]

                for kt in range(nkt):
                    jo = kt - KTW * w
                    st = ps_st.tile([P, G, WQ], F32, tag="st")
                    for h in (0, 1):
                        ph = ds(HD * h, HD)
                        nc.tensor.matmul(
                            st[:, h, :],
                            KT[ph, ds(kt * P, P)], QT[ph, qsl],
                            start=True, stop=True,
                            tile_position=(HD * h, 0),
                        )
                        if jo >= 0:  # diagonal tile: triangular causal mask
                            nc.vector.tensor_add(
                                st[:, h, ds(P * jo, P)],
                                st[:, h, ds(P * jo, P)], tri_sb[:],
                            )
                    pt = ptpool.tile([P, G, WQ], F32R, tag="pt")
                    nc.scalar.activation(
                        pt[:], st[:],
                        mybir.ActivationFunctionType.Exp, scale=SCALE,
                    )
                    # columns < 128*jo are fully masked: trim them out of
                    # the ctx matmul instead of zeroing PT
                    coff = P * jo if jo > 0 else 0
                    for h in (0, 1):
                        nc.tensor.matmul(
                            ctxs[h][0 : HD + 1, coff:WQ],
                            V65[:, h, kt, :], pt[:, h, coff:WQ],
                            start=(kt == 0), stop=(kt == nkt - 1),
                        )

                # normalize: ctxn_h[d, q] = ctx_h[d, q] * (1/den_h[q])
                recip = miscp.tile([P, 2, WQ], F32R, tag="recip")
                with nc.allow_low_precision(reason="fp32r has fp32 width"):
                    for h in (0, 1):
                        nc.vector.reciprocal(
                            recip[HD : HD + 1, h, :], ctxs[h][HD : HD + 1, :]
                        )
                ctxns = []
                for h in (0, 1):
                    bc = ps_st.tile([P, G, WQ], F32, tag="st")
                    nc.tensor.matmul(
                        bc[0:HD, 0, :],
                        ones_sb[HD : HD + 1, 0:HD], recip[HD : HD + 1, h, :],
                        start=True, stop=True,
                        tile_position=(HD, 0),
                    )
                    bcs = miscp.tile([HD, WQ], F32, tag=f"bcs{h}")
                    nc.vector.tensor_copy(bcs[:], bc[0:HD, 0, :])
                    ctxn = ctxpool.tile([HD, WQ], F32R, tag=f"ctxn{h}")
                    ctxns.append(ctxn)
                    nc.vector.tensor_mul(ctxn[:], ctxs[h][0:HD, :], bcs[:])

                # out-projection for this window's 4 s-tiles
                for t4 in range(KTW):
                    ysb = ypool.tile([P, D], F32, tag="ysb")
                    for oc in range(2):
                        yps = ps_st.tile([P, G, WQ], F32, tag="st")
                        nc.tensor.matmul(
                            yps[:, 0, :],
                            ctxns[0][:, ds(t4 * P, P)], wo0[:, ds(oc * WQ, WQ)],
                            start=True, stop=False,
                        )
                        nc.tensor.matmul(
                            yps[:, 0, :],
                            ctxns[1][:, ds(t4 * P, P)], wo1[:, ds(oc * WQ, WQ)],
                            start=False, stop=True,
                        )
                        nc.vector.tensor_copy(ysb[:, ds(oc * WQ, WQ)], yps[:, 0, :])
                    nc.sync.dma_start(y.ap()[ds(w * WQ + t4 * P, P), :], ysb[:])

            # ---- software-pipelined emission ----
            qkv_chunk(0)
            qkv_chunk(1)
            vtrans_chunk(0)
            for w in range(NW):
                if w + 2 < NW:
                    qkv_chunk(w + 2)
                if w + 1 < NW:
                    vtrans_chunk(w + 1)
                attention_window(w)

    nc.compile()
    return nc


def _get_nc():
    global _CACHED_NC
    if _CACHED_NC is None:
        _CACHED_NC = _build()
    return _CACHED_NC


def kernel(x, Wq, Wk, Wv, Wo, bo):
    x = np.asarray(x, dtype=np.float32)
    Wq = np.asarray(Wq, dtype=np.float32)
    Wk = np.asarray(Wk, dtype=np.float32)
    Wv = np.asarray(Wv, dtype=np.float32)
    Wo = np.asarray(Wo, dtype=np.float32)
    bo = np.asarray(bo, dtype=np.float32)

    xT = np.ascontiguousarray(x.reshape(S, D).T)
    col = np.arange(P)
    tri = np.where(col[None, :] >= col[:, None], 0.0, NEG).astype(np.float32)
    ident = np.eye(P, dtype=np.float32)

    in_maps = []
    for c in range(NC):
        dsl = slice(P * c, P * (c + 1))
        in_maps.append({
            "xT": xT,
            "wq": np.ascontiguousarray(Wq[dsl, :].T),
            "wk": np.ascontiguousarray(Wk[dsl, :].T),
            "wv": np.ascontiguousarray(Wv[dsl, :].T),
            "wo": np.ascontiguousarray(Wo[:, dsl].T),
            "tri": tri,
            "ident": ident,
        })

    nc = _get_nc()
    res = run_bass_kernel_spmd(nc, in_maps, core_ids=list(range(NC)))
    out = np.zeros((S, D), dtype=np.float32)
    for c in range(NC):
        out += res.results[c]["y"]
    out += bo[None, :]
    return out.reshape(1, S, D)
